# revision 21
# baseline (speedup 1.0000x reference)
"""A3C loss kernel for Trainium2 (8 NeuronCores, data-parallel over batch).

The reference is a reverse scan over T=128 timesteps per trajectory:
    R_t   = sum_{s>=t} g^(s-t) r_s + g^(T-t) R0
    gae_t telescopes to adv_t = R_t - v_t   (lambda=1 GAE)
    critic = 0.5 * sum_t adv_t^2
    actor  = -sum_t lp_t * adv_t - beta * sum_{t,a} ent
The suffix scan is a matmul with a [T,T] discount matrix, so the loss is
DMA + one A-reduction + transpose + one matmul per 128-row block.

Layout (streams at the ~435 GB/s SBUF-fabric ceiling, 75.6 MB/core):
  - values/rewards preloaded whole as [128, 64, 128] tiles; partition p
    holds rows [64p, 64p+64) so each partition line is 32KB contiguous
    DRAM (per-block [128,128] loads would be 512B lines at ~72% of line
    rate).
  - gamma*R0 is folded into rewards[:, :, T-1] with a single strided
    gpsimd op instead of 64 per-block ops.
  - the identity and discount-matrix constants are built on-chip
    (iota/affine_select/Exp) during the preload, keeping 128KB of
    512B-line packets out of the port-bound DMA queue.
  - log_probs/entropies stream in G=4-block chunks (16KB/partition
    contiguous) to quarter dma_start count (each start costs 16 4B
    sem-inc packets).
  - every engine is kept under the ~4.9us/chunk DMA pace so buffer
    recycling never throttles the stream: the A=8 log_prob reduction
    runs as a pairwise tensor_tensor tree on the otherwise-idle gpsimd
    engine, the critic square-accumulate as a vector STT, and the
    entropy sums on the scalar engine.
  - transpose/matmul/adv/critic depend only on values/rewards, so they
    run several chunks ahead of the lp/en stream; only the A-reduction,
    the actor dot and the entropy sum sit on the streaming tail.
  - outputs staged per 16-block group and stored on the scalar HWDGE
    ring, so only the last group's store trails the final chunk.
  - the last chunk's loads are split per block so the drain after
    the final HBM byte is one block deep, not one chunk.
"""

import numpy as np
from contextlib import ExitStack

import concourse.bacc as bacc
import concourse.bass as bass
import concourse.tile as tile
from concourse import mybir
from concourse.bass_utils import run_bass_kernel_spmd

GAMMA = 0.99
BETA = 0.01
B, T, A = 65536, 128, 8
N_CORES = 8
BC = B // N_CORES

F32 = mybir.dt.float32
ALU = mybir.AluOpType
ACTF = mybir.ActivationFunctionType


def _discount_matrix() -> np.ndarray:
    # L[s, t] = gamma^(s-t) for s >= t else 0
    s = np.arange(T, dtype=np.float64)[:, None]
    t = np.arange(T, dtype=np.float64)[None, :]
    m = np.where(s >= t, GAMMA ** np.maximum(s - t, 0.0), 0.0)
    return m.astype(np.float32)


def _blk(t3, k):
    """[128, kb, T] tile -> [128, T] view of block k."""
    try:
        return t3[:, k, :]
    except Exception:
        return t3[:, k : k + 1, :].squeeze(1)


def build_nc(bc: int = BC):
    kb = bc // 128
    assert bc % 128 == 0
    G = 4                 # blocks per streamed lp/en chunk
    nch = kb // G         # chunks
    SPLIT = 1             # last chunk streams per-block to shorten the drain
    GRP = 16              # blocks per output store group
    ngrp = kb // GRP

    nc = bacc.Bacc("TRN2", target_bir_lowering=False, debug=False)

    v_d = nc.dram_tensor("values", [bc, T], F32, kind="ExternalInput")
    lv_d = nc.dram_tensor("last_value", [bc], F32, kind="ExternalInput")
    r_d = nc.dram_tensor("rewards", [bc, T], F32, kind="ExternalInput")
    lp_d = nc.dram_tensor("log_probs", [bc, T, A], F32, kind="ExternalInput")
    en_d = nc.dram_tensor("entropies", [bc, T, A], F32, kind="ExternalInput")
    tm_d = nc.dram_tensor("terminal_mask", [bc], mybir.dt.uint8, kind="ExternalInput")
    out_d = nc.dram_tensor("out", [bc, 2], F32, kind="ExternalOutput")

    # partition p owns rows [kb*p, kb*(p+1)): contiguous DRAM per partition
    v3 = v_d.rearrange("(p k) t -> p k t", k=kb)
    r3 = r_d.rearrange("(p k) t -> p k t", k=kb)
    lp4 = lp_d.rearrange("(p c g) t a -> c p (g t) a", c=nch, g=G)
    en3 = en_d.rearrange("(p c g) t a -> c p (g t a)", c=nch, g=G)
    lv_view = lv_d.rearrange("(p k) -> p k", k=kb)
    tm_view = tm_d.rearrange("(p k) -> p k", k=kb)
    out2 = out_d.rearrange("(p k) j -> p (k j)", k=kb)

    with tile.TileContext(nc) as tc, ExitStack() as ctx:
        singles = ctx.enter_context(tc.tile_pool(name="singles", bufs=1))
        rtp = ctx.enter_context(tc.tile_pool(name="rtp", bufs=6))
        advp = ctx.enter_context(tc.tile_pool(name="advp", bufs=16))
        scrp = ctx.enter_context(tc.tile_pool(name="scrp", bufs=2))
        lp2p = ctx.enter_context(tc.tile_pool(name="lp2p", bufs=3))
        lpp = ctx.enter_context(tc.tile_pool(name="lpp", bufs=3))
        enp = ctx.enter_context(tc.tile_pool(name="enp", bufs=3))
        escp = ctx.enter_context(tc.tile_pool(name="escp", bufs=2))
        s1p = ctx.enter_context(tc.tile_pool(name="s1p", bufs=1))
        s2p = ctx.enter_context(tc.tile_pool(name="s2p", bufs=1))
        psA = ctx.enter_context(tc.tile_pool(name="psA", bufs=3, space="PSUM"))
        psB = ctx.enter_context(tc.tile_pool(name="psB", bufs=3, space="PSUM"))

        # singles go through SWDGE (gpsimd) so the SP HWDGE FIFO starts on
        # the big loads immediately
        lv_s = singles.tile([128, kb], F32)
        nc.gpsimd.dma_start(out=lv_s, in_=lv_view)
        tm_s = singles.tile([128, kb], mybir.dt.uint8)
        nc.gpsimd.dma_start(out=tm_s, in_=tm_view)

        # SP HWDGE queue order = consumption order: rewards, values, then
        # the lp/en stream.
        rfull = singles.tile([128, kb, T], F32)
        nc.sync.dma_start(out=rfull, in_=r3)
        vfull = singles.tile([128, kb, T], F32)
        nc.sync.dma_start(out=vfull, in_=v3)

        # constants built on-chip while the preloads stream (the engines
        # are idle then, and this keeps 128KB of 512B-line packets out of
        # the DMA queue): iden[p,x] = (x == p); lgam[s,t] = gamma^(s-t)*(s>=t)
        iden_s = singles.tile([128, 128], F32)
        ones = singles.tile([128, 128], F32)
        nc.vector.memset(ones, 1.0)
        nc.gpsimd.affine_select(
            out=iden_s, in_=ones, pattern=[[1, 128]], base=0,
            channel_multiplier=-1, compare_op=ALU.is_equal, fill=0.0,
        )
        smt = singles.tile([128, 128], mybir.dt.int32)
        nc.gpsimd.iota(smt, pattern=[[-1, 128]], base=0, channel_multiplier=1)
        smtf = singles.tile([128, 128], F32)
        nc.gpsimd.tensor_copy(out=smtf, in_=smt)
        nc.gpsimd.tensor_scalar_mul(smtf, smtf, float(np.log(GAMMA)))
        expf = singles.tile([128, 128], F32)
        nc.scalar.activation(
            out=expf, in_=smtf, func=ACTF.Exp, bias=0.0, scale=1.0
        )
        lgam_s = singles.tile([128, 128], F32)
        nc.gpsimd.affine_select(
            out=lgam_s, in_=expf, pattern=[[-1, 128]], base=0,
            channel_multiplier=1, compare_op=ALU.is_ge, fill=0.0,
        )

        # gr0 = gamma * last_value * (1 - mask)
        tmf = singles.tile([128, kb], F32)
        nc.gpsimd.tensor_copy(out=tmf, in_=tm_s)
        lvm = singles.tile([128, kb], F32)
        nc.gpsimd.tensor_mul(lvm, lv_s, tmf)
        gr0 = singles.tile([128, kb], F32)
        nc.gpsimd.tensor_sub(gr0, lv_s, lvm)
        nc.gpsimd.tensor_scalar_mul(gr0, gr0, GAMMA)

        # fold gamma*R0 into the last timestep of every block at once
        nc.gpsimd.tensor_tensor(
            out=rfull[:, :, T - 1 : T],
            in0=rfull[:, :, T - 1 : T],
            in1=gr0.unsqueeze(2),
            op=ALU.add,
        )

        stage = [
            singles.tile([128, 2 * GRP], F32, name=f"stage{i}") for i in range(ngrp)
        ]
        accs = [singles.tile([128, GRP], F32, name=f"acc{i}") for i in range(ngrp)]
        nbes = [singles.tile([128, GRP], F32, name=f"nbe{i}") for i in range(ngrp)]

        advs = [None] * kb

        def early(c):
            # depends only on rewards/values: runs ahead of the lp/en stream
            for g in range(G):
                k = c * G + g
                si, j = k // GRP, k % GRP
                trp = psA.tile([128, 128], F32)
                nc.tensor.transpose(trp, _blk(rfull, k), iden_s)
                rT = rtp.tile([128, 128], F32)
                nc.vector.tensor_copy(out=rT, in_=trp)
                # R[b, t] = sum_s r'T[s, b] * Lgam[s, t]
                Rp = psB.tile([128, 128], F32)
                nc.tensor.matmul(Rp, lhsT=rT, rhs=lgam_s, start=True, stop=True)
                adv = advp.tile([128, 128], F32)
                nc.vector.tensor_sub(adv, Rp, _blk(vfull, k))
                advs[k] = adv
                # critic = 0.5 * sum_t adv^2  (as (0.5*adv)*adv with accum,
                # keeping the scalar engine free for the entropy sums)
                sq = scrp.tile([128, 128], F32)
                nc.vector.scalar_tensor_tensor(
                    out=sq, in0=adv, scalar=0.5, in1=adv,
                    op0=ALU.mult, op1=ALU.mult,
                    accum_out=stage[si][:, 2 * j + 1 : 2 * j + 2],
                )

        def stream(c):
            split = c >= nch - SPLIT
            lpb = lpp.tile([128, G * T, A], F32)
            enb = enp.tile([128, G * T * A], F32)
            if split:
                for g in range(G):
                    nc.sync.dma_start(
                        out=lpb[:, g * T : (g + 1) * T, :],
                        in_=lp4[c][:, g * T : (g + 1) * T, :],
                    )
                    nc.sync.dma_start(
                        out=enb[:, g * T * A : (g + 1) * T * A],
                        in_=en3[c][:, g * T * A : (g + 1) * T * A],
                    )
            else:
                nc.sync.dma_start(out=lpb, in_=lp4[c])
                nc.sync.dma_start(out=enb, in_=en3[c])

            # lp2[b, (g t)] = sum_a log_probs.  Steady state: pairwise tree on
            # the otherwise-idle gpsimd engine (its tensor_reduce can't do
            # free-axis X, but tensor_tensor on strided slices can).  Split
            # chunks at the drain use the vector reduce per block instead,
            # overlapping the preceding half's DMA.
            lp2 = lp2p.tile([128, G * T], F32)
            if split:
                for g in range(G):
                    nc.vector.reduce_sum(
                        out=lp2[:, g * T : (g + 1) * T],
                        in_=lpb[:, g * T : (g + 1) * T, :],
                        axis=mybir.AxisListType.X,
                    )
            else:
                s1 = s1p.tile([128, G * T, 4], F32)
                nc.gpsimd.tensor_tensor(
                    out=s1, in0=lpb[:, :, 0:4], in1=lpb[:, :, 4:8], op=ALU.add
                )
                s2 = s2p.tile([128, G * T, 2], F32)
                nc.gpsimd.tensor_tensor(
                    out=s2, in0=s1[:, :, 0:2], in1=s1[:, :, 2:4], op=ALU.add
                )
                nc.gpsimd.tensor_tensor(
                    out=lp2.unsqueeze(2), in0=s2[:, :, 0:1], in1=s2[:, :, 1:2],
                    op=ALU.add,
                )

            for g in range(G):
                k = c * G + g
                si, j = k // GRP, k % GRP
                # nbe[b] = -beta * sum_{t,a} entropies
                esc = escp.tile([128, T * A], F32)
                nc.scalar.activation(
                    out=esc, in_=enb[:, g * T * A : (g + 1) * T * A],
                    func=ACTF.Copy, bias=0.0, scale=-BETA,
                    accum_out=nbes[si][:, j : j + 1],
                )
                # actor partial: -sum_t lp*adv
                prod = scrp.tile([128, 128], F32)
                nc.vector.scalar_tensor_tensor(
                    out=prod, in0=advs[k], scalar=-1.0,
                    in1=lp2[:, g * T : (g + 1) * T],
                    op0=ALU.mult, op1=ALU.mult,
                    accum_out=accs[si][:, j : j + 1],
                )

            if (c + 1) % (GRP // G) == 0:
                si = (c + 1) // (GRP // G) - 1
                # actor = acc + nbe, interleaved into the staging tile
                st3 = stage[si].rearrange("p (j two) -> p j two", two=2)
                nc.vector.tensor_tensor(
                    out=st3[:, :, 0:1], in0=accs[si].unsqueeze(2),
                    in1=nbes[si].unsqueeze(2), op=ALU.add,
                )
                # store this group on the second HWDGE ring, out of the
                # SP FIFO that carries the input loads
                nc.scalar.dma_start(
                    out=out2[:, si * 2 * GRP : (si + 1) * 2 * GRP],
                    in_=stage[si],
                )

        LAG = 3
        for i in range(nch + LAG):
            if i < nch:
                early(i)
            if i >= LAG:
                stream(i - LAG)

    nc.compile()
    return nc


_NC = None


def _get_nc():
    global _NC
    if _NC is None:
        _NC = build_nc(BC)
    return _NC


def _make_in_maps(inputs: dict) -> list[dict]:
    v = np.ascontiguousarray(np.asarray(inputs["values"], dtype=np.float32))
    lv = np.ascontiguousarray(np.asarray(inputs["last_value"], dtype=np.float32))
    r = np.ascontiguousarray(np.asarray(inputs["rewards"], dtype=np.float32))
    lp = np.ascontiguousarray(np.asarray(inputs["log_probs"], dtype=np.float32))
    en = np.ascontiguousarray(np.asarray(inputs["entropies"], dtype=np.float32))
    tm = np.ascontiguousarray(np.asarray(inputs["terminal_mask"]).astype(np.uint8))
    maps = []
    for c in range(N_CORES):
        sl = slice(c * BC, (c + 1) * BC)
        maps.append(
            {
                "values": v[sl],
                "last_value": lv[sl],
                "rewards": r[sl],
                "log_probs": lp[sl],
                "entropies": en[sl],
                "terminal_mask": tm[sl],
            }
        )
    return maps


def _run(inputs: dict, trace: bool = False):
    nc = _get_nc()
    res = run_bass_kernel_spmd(
        nc,
        _make_in_maps(inputs),
        core_ids=list(range(N_CORES)),
        trace=trace,
    )
    out = np.concatenate([res.results[c]["out"] for c in range(N_CORES)], axis=0)
    return out, res


def kernel(**inputs) -> np.ndarray:
    out, _ = _run(inputs, trace=False)
    return out
